# revision 23
# baseline (speedup 1.0000x reference)
"""Trainium2 Bass kernel: multi-head cross attention (B=2, S=2048, D=1024, H=16).

Sharding: 8 cores = 2 batches x 4 head-groups (Megatron style).
Each core computes, for its batch b and its 4 heads (columns g*256..g*256+255
of the QKV projections / rows of the O projection):

    qT = (qW_g^T y_b^T + qB_g)          [256, 2048]  (head-dim on partitions)
    kT = (kW_g^T X_b^T + kB_g)          [256, 2048]
    v  = (X_b vW_g)                     [2048, 256]  (kv on partitions)
    per head h, kv-chunk c:  S^T[c] = kT_h[:,c]^T qT_h   (kv on partitions)
    P = exp(S^T / 8)  (softmax w/o max subtraction -- scores are O(5), safe)
    fused:  [O^T_h ; Z_h] = sum_c [V_h[c] | 1]^T P_h[c]   (ones-augmented V:
            one M=128 matmul per (chunk, head) yields the attnV accumulation
            in rows 0:64 and the softmax denominator, 64x replicated, in
            rows 64:128 of one PSUM bank)
    OT = (O^T / Z) via DVE reciprocal+mul with partition-shifted reads
    out_partial = OT^T oW_g

Host sums the 4 partials per batch and adds (vB @ oW + oB).

HW notes (measured in-context this session; the device has at least two
clock states, so absolute ns vary ~1.7x between runs -- only same-process
comparisons are meaningful):
  - single K=128 matmul chains beat row-tiled half-K "concurrent" pairs
    ~2.9x per contraction step (no dual-issue observed in-context); all
    projections use single chains, one PSUM bank each;
  - exp: ACTIVATE N=1024 from PSUM f32 is the sweet spot (N=2048 spanning
    4 banks is disproportionately slow; SBUF-sourced exp is erratic);
  - PE matmuls and ACT activations serialize almost fully on HW (measured
    with dependency-free interleavings), so the attention inner loop costs
    ~(S pair + 2 fused CD matmuls + exp) per chunk; keeping instruction
    count low matters more than nominal engine overlap;
  - S stays a row-tiled K=64 pair (a zero-padded K=128 variant measured
    slower in-context); S is double-buffered in PSUM so exp(c) overlaps
    the S(c+1) matmuls at the queue level;
  - the 14-pair PE warmup block measurably helps (clock-gate state).

PSUM banks (8): S double-buffer 2x[128,1024] (4) + fused CD accumulators
CA/CB (2) + single-bank projection filler chains (2).

All matmuls in bf16 (fp32 PSUM accumulation); softmax/normalization in fp32.
"""

import sys
from collections import deque

import numpy as np

sys.path.insert(0, "/opt/trn_rl_repo")

import concourse.bass as bass  # noqa: E402
import concourse.bacc as bacc  # noqa: E402
import concourse.mybir as mybir  # noqa: E402
import concourse.tile as tile  # noqa: E402

F32 = mybir.dt.float32
BF16 = mybir.dt.bfloat16
EXP = mybir.ActivationFunctionType.Exp
ADD = mybir.AluOpType.add

D = 1024          # d_model
SQ = 2048         # query length
SKV = 2048        # kv length
CPC = 256         # projection columns per core (4 heads x 64)
NK = D // 128     # 8 contraction chunks
NQB = SQ // 512   # 4 query blocks
NC_ = SKV // 128  # 16 kv chunks
N_CORES = 8

# timing-ablation flags (timing builds only -- results become garbage):
#   cheapproj : projections emit memset instead of matmul chains
#   cheapexp  : exp replaced by a DVE copy (removes ACT work)
#   nosum     : no denominator matmuls / no normalize (OT = copy of C)
#   cheapoproj: output projection emits memset + DMA only
ABLATE = set()

# S-matmul strategy: "ktz" = lone K=128 chains against zero-padded per-head
# kT tiles; "pair" = row-tiled half-K pairs on a shared kT tile.
# Measured in-context: pair 249us vs ktz 272us -> pair wins.
S_MODE = "pair"

# attnV strategy: "fused" = one M=128 matmul per (chunk, head) against
# ones-augmented V ([V_h | 1]) producing C rows 0:64 and the softmax
# denominator (64x replicated) rows 64:128 in one pass; "colpair" = separate
# M=64 col-tiled pairs for C and D.
CD_MODE = "fused"

# emit the PE warmup block at the top of compute()
WARMUP = True


def build_program(loop_n=None):
    """Build and compile the single-core SPMD program. Returns nc.

    loop_n: if set (>1), wrap the whole kernel body in a hardware loop that
    repeats it loop_n times -- used only for wall-clock timing (amortizes the
    host/axon dispatch overhead).
    """
    nc = bacc.Bacc(
        "TRN2",
        target_bir_lowering=False,
        debug=False,
        enable_asserts=True,
        num_devices=N_CORES,
    )

    xt_d = nc.dram_tensor("xt", [D, SKV], BF16, kind="ExternalInput").ap()
    yt_d = nc.dram_tensor("yt", [D, SQ], BF16, kind="ExternalInput").ap()
    qw_d = nc.dram_tensor("qw", [D, CPC], BF16, kind="ExternalInput").ap()
    kw_d = nc.dram_tensor("kw", [D, CPC], BF16, kind="ExternalInput").ap()
    vw_d = nc.dram_tensor("vw", [D, CPC], BF16, kind="ExternalInput").ap()
    ow_d = nc.dram_tensor("ow", [CPC, D], BF16, kind="ExternalInput").ap()
    qb_d = nc.dram_tensor("qbias", [CPC], F32, kind="ExternalInput").ap()
    kb_d = nc.dram_tensor("kbias", [CPC], F32, kind="ExternalInput").ap()
    out_d = nc.dram_tensor("out", [SQ, D], F32, kind="ExternalOutput").ap()

    with tile.TileContext(nc) as tc:
        if loop_n and loop_n > 1:
            # timing mode: load inputs once, loop the compute body so the
            # per-iteration wall time isolates compute (the For_i back edge
            # is a full barrier anyway)
            st = _Stage(tc, nc, xt_d, yt_d, qw_d, kw_d, vw_d, ow_d,
                        qb_d, kb_d, out_d)
            st.load()
            with tc.For_i(0, loop_n, 1):
                st.compute()
            st.close()
        else:
            st = _Stage(tc, nc, xt_d, yt_d, qw_d, kw_d, vw_d, ow_d,
                        qb_d, kb_d, out_d)
            st.load()
            st.compute()
            st.close()

    nc.compile()
    return nc


class Feeder:
    """Queue of emission generators; pop(n) advances by n yield-steps."""

    def __init__(self):
        self.gens = deque()

    def add(self, g):
        self.gens.append(g)

    def run_all(self, g):
        for _ in g:
            pass

    def pop(self, budget=1):
        while budget > 0 and self.gens:
            try:
                next(self.gens[0])
                budget -= 1
            except StopIteration:
                self.gens.popleft()

    def drain(self):
        while self.gens:
            self.pop(1000)


class _Stage:
    """Kernel emission split into load() (input DMAs, persistent tiles) and
    compute() (everything else), so timing builds can loop compute only."""

    def __init__(self, tc, nc, xt_d, yt_d, qw_d, kw_d, vw_d, ow_d,
                 qb_d, kb_d, out_d):
        from contextlib import ExitStack
        self.tc, self.nc = tc, nc
        self.xt_d, self.yt_d = xt_d, yt_d
        self.qw_d, self.kw_d, self.vw_d, self.ow_d = qw_d, kw_d, vw_d, ow_d
        self.qb_d, self.kb_d, self.out_d = qb_d, kb_d, out_d
        self.ctx = ExitStack()

    def close(self):
        self.ctx.close()

    def load(self):
        tc, nc, ctx = self.tc, self.nc, self.ctx
        self.pers = ctx.enter_context(tc.tile_pool(name="pers", bufs=1))
        self.p_pool = ctx.enter_context(tc.tile_pool(name="ppool", bufs=3))
        self.r_pool = ctx.enter_context(tc.tile_pool(name="rpool", bufs=2))
        self.ot_pool = ctx.enter_context(tc.tile_pool(name="otpool", bufs=4))
        self.oe_pool = ctx.enter_context(tc.tile_pool(name="oepool", bufs=3))
        self.tmp_pool = ctx.enter_context(tc.tile_pool(name="tmppool", bufs=2))
        # PSUM budget (8 banks): S double-buffered 2x[128,1024] (4) takes exp
        # off the S-matmul critical path; attnV accumulator C as a single-bank
        # col-tiled pair (1); denominator Dt (1); half-K filler chains (2)
        self.s_pool = ctx.enter_context(
            tc.tile_pool(name="spool", bufs=2, space="PSUM"))
        self.c_pool = ctx.enter_context(
            tc.tile_pool(name="cpool", bufs=1, space="PSUM"))
        self.d_pool = ctx.enter_context(
            tc.tile_pool(name="dpool", bufs=1, space="PSUM"))
        self.cd_pool = ctx.enter_context(
            tc.tile_pool(name="cdpool", bufs=2, space="PSUM"))
        pers = self.pers

        # load order tuned for the one-shot ramp: biases + kW first, then the
        # xT column block the first kT chains read (nb0), then vW (v prefix),
        # then yT nb0 + qW (first qT), then the remaining xT/yT blocks in
        # feeder-consumption order, oW last.  Column-split DMAs let the
        # prefix chains start after ~1.5MB instead of ~5MB of input traffic.
        self.qb_sb = pers.tile([128, 2], F32, tag="qb", name="qb_sb")
        nc.sync.dma_start(self.qb_sb[:], self.qb_d.rearrange("(a p) -> p a", p=128))
        self.kb_sb = pers.tile([128, 2], F32, tag="kb", name="kb_sb")
        nc.sync.dma_start(self.kb_sb[:], self.kb_d.rearrange("(a p) -> p a", p=128))
        self.kw_sb, self.qw_sb, self.vw_sb = [], [], []
        for k in range(NK):
            kwt = pers.tile([128, CPC], BF16, tag=f"kw{k}", name=f"kw{k}")
            nc.sync.dma_start(kwt[:], self.kw_d[k * 128:(k + 1) * 128, :])
            self.kw_sb.append(kwt)
        self.xt = [pers.tile([128, SKV], BF16, tag=f"xt{k}", name=f"xt{k}")
                   for k in range(NK)]
        for k in range(NK):
            nc.sync.dma_start(self.xt[k][:, 0:512],
                              self.xt_d[k * 128:(k + 1) * 128, 0:512])
        for k in range(NK):
            vwt = pers.tile([128, CPC], BF16, tag=f"vw{k}", name=f"vw{k}")
            nc.sync.dma_start(vwt[:], self.vw_d[k * 128:(k + 1) * 128, :])
            self.vw_sb.append(vwt)
        self.yt = [pers.tile([128, SQ], BF16, tag=f"yt{k}", name=f"yt{k}")
                   for k in range(NK)]
        for k in range(NK):
            nc.sync.dma_start(self.yt[k][:, 0:512],
                              self.yt_d[k * 128:(k + 1) * 128, 0:512])
        for k in range(NK):
            qwt = pers.tile([128, CPC], BF16, tag=f"qw{k}", name=f"qw{k}")
            nc.sync.dma_start(qwt[:], self.qw_d[k * 128:(k + 1) * 128, :])
            self.qw_sb.append(qwt)
        for nb in range(1, 4):
            nsl = slice(nb * 512, (nb + 1) * 512)
            for k in range(NK):
                nc.sync.dma_start(self.xt[k][:, nsl],
                                  self.xt_d[k * 128:(k + 1) * 128, nsl])
        for nb in range(1, 4):
            nsl = slice(nb * 512, (nb + 1) * 512)
            for k in range(NK):
                nc.sync.dma_start(self.yt[k][:, nsl],
                                  self.yt_d[k * 128:(k + 1) * 128, nsl])
        self.ow_sb = []
        for p in range(2):
            owt = pers.tile([128, D], BF16, tag=f"ow{p}", name=f"ow{p}")
            nc.sync.dma_start(owt[:], self.ow_d[p * 128:(p + 1) * 128, :])
            self.ow_sb.append(owt)

        self.ones = pers.tile([128, 64], BF16, tag="ones", name="ones")
        nc.vector.memset(self.ones[:], 1.0)
        self.wu = pers.tile([128, 512], BF16, tag="wu", name="wu")
        nc.vector.memset(self.wu[:], 0.001)
        # prefetch the exp table set (~2.7us ACT_TABLE_LOAD) under the input
        # DMA wait so the first real exp doesn't pay it
        actw = pers.tile([128, 2], F32, tag="actw", name="actw")
        nc.vector.memset(actw[:], 0.0)
        actw2 = pers.tile([128, 2], BF16, tag="actw2", name="actw2")
        nc.scalar.activation(actw2[:], actw[:], EXP, scale=1.0)

        # Per-head zero-padded kT tiles: ktz[pair][0] has head-A dims in
        # rows 0:64 and zeros in rows 64:128 (ktz[pair][1] mirrored), so each
        # S matmul is a lone K=128 chain (measured ~2.9x faster per step than
        # the row-tiled half-K pair) -- the zero rows contribute nothing.
        if S_MODE == "ktz":
            self.ktz = [[pers.tile([128, SKV], BF16, tag=f"ktz{p}{h}",
                                   name=f"ktz{p}{h}") for h in range(2)]
                        for p in range(2)]
            for p in range(2):
                nc.vector.memset(self.ktz[p][0][64:128, :], 0.0)
                nc.vector.memset(self.ktz[p][1][0:64, :], 0.0)
        else:
            self.kt = [pers.tile([128, SKV], BF16, tag=f"kt{p}",
                                 name=f"kt{p}") for p in range(2)]
        self.qt = [pers.tile([128, SQ], BF16, tag=f"qt{p}", name=f"qt{p}")
                   for p in range(2)]
        if CD_MODE == "fused":
            # per (chunk, head) 128-col blocks: [V_h (64) | ones (64)];
            # memset everything to 1.0 once, v_gen overwrites the V halves
            self.v_sb = pers.tile([128, NC_ * CPC * 2], BF16, tag="v",
                                  name="v_sb")
            nc.vector.memset(self.v_sb[:], 1.0)
        else:
            self.v_sb = pers.tile([128, NC_ * CPC], BF16, tag="v",
                                  name="v_sb")

    # ---- projection emitters: single K=128 matmul chains (measured ~2.9x
    #      faster per contraction step than row-tiled half-K pairs), one
    #      PSUM bank per chain, bias-add fused into the PSUM->SBUF move ----
    def proj_gen(self, w_tiles, x_tiles, colsl, xsl, n, dest, bias, unm):
        nc, cd_pool = self.nc, self.cd_pool
        if "cheapproj" in ABLATE:
            nc.vector.memset(dest, 0.01)
            yield
            return
        psA = cd_pool.tile([128, 512], F32, tag="cd", name=f"{unm}A")
        for k in range(NK):
            st, sp = (k == 0), (k == NK - 1)
            nc.tensor.matmul(psA[:, 0:n], w_tiles[k][:, colsl],
                             x_tiles[k][:, xsl], start=st, stop=sp)
            if k % 2 == 1 and k < NK - 1:
                yield
        if bias is None:
            nc.vector.tensor_copy(dest, psA[:, 0:n])
        else:
            nc.vector.tensor_scalar_add(dest, psA[:, 0:n], bias)
        yield

    def kt_gen(self, pair, nb):
        nc, cd_pool = self.nc, self.cd_pool
        sl = slice(pair * 128, (pair + 1) * 128)
        nsl = slice(nb * 512, (nb + 1) * 512)
        if S_MODE != "ktz":
            return self.proj_gen(self.kw_sb, self.xt, sl, nsl, 512,
                                 self.kt[pair][:, nsl],
                                 self.kb_sb[:, pair:pair + 1],
                                 f"ktp{pair}_{nb}")
        return self._kt_gen_ktz(pair, nsl, sl)

    def _kt_gen_ktz(self, pair, nsl, sl):
        nc, cd_pool = self.nc, self.cd_pool
        ktzA, ktzB = self.ktz[pair]
        if "cheapproj" in ABLATE:
            nc.vector.memset(ktzA[0:64, nsl], 0.01)
            nc.vector.memset(ktzB[64:128, nsl], 0.01)
            yield
            return
        psA = cd_pool.tile([128, 512], F32, tag="cd",
                           name=f"ktp{pair}_{nsl.start}")
        for k in range(NK):
            st, sp = (k == 0), (k == NK - 1)
            nc.tensor.matmul(psA[:], self.kw_sb[k][:, sl],
                             self.xt[k][:, nsl], start=st, stop=sp)
            if k % 2 == 1 and k < NK - 1:
                yield
        # split the pair's 128 projection rows into the two zero-padded
        # per-head tiles (bias fused)
        nc.vector.tensor_scalar_add(ktzA[0:64, nsl], psA[0:64, :],
                                    self.kb_sb[0:64, pair:pair + 1])
        nc.vector.tensor_scalar_add(ktzB[64:128, nsl], psA[64:128, :],
                                    self.kb_sb[64:128, pair:pair + 1])
        yield

    def qt_gen(self, pair, qb):
        sl = slice(pair * 128, (pair + 1) * 128)
        nsl = slice(qb * 512, (qb + 1) * 512)
        return self.proj_gen(self.qw_sb, self.yt, sl, nsl, 512,
                             self.qt[pair][:, nsl],
                             self.qb_sb[:, pair:pair + 1], f"qtp{pair}_{qb}")

    def v_gen(self, s):
        ssl = slice(s * 128, (s + 1) * 128)
        if CD_MODE != "fused":
            return self.proj_gen(self.xt, self.vw_sb, ssl, slice(0, CPC),
                                 CPC, self.v_sb[:, s * CPC:(s + 1) * CPC],
                                 None, f"vp{s}")
        return self._v_gen_fused(s, ssl)

    def _v_gen_fused(self, s, ssl):
        nc, cd_pool = self.nc, self.cd_pool
        base = s * CPC * 2
        if "cheapproj" in ABLATE:
            for h in range(4):
                nc.vector.memset(
                    self.v_sb[:, base + h * 128:base + h * 128 + 64], 0.01)
            yield
            return
        psA = cd_pool.tile([128, 512], F32, tag="cd", name=f"vp{s}")
        for k in range(NK):
            st, sp = (k == 0), (k == NK - 1)
            nc.tensor.matmul(psA[:, 0:CPC], self.xt[k][:, ssl],
                             self.vw_sb[k][:, 0:CPC], start=st, stop=sp)
            if k % 2 == 1 and k < NK - 1:
                yield
        # scatter the 4 heads' 64-col blocks into the ones-augmented layout
        # with one strided copy (dest skips each block's ones half)
        dst = self.v_sb[:, base:base + 512].rearrange(
            "p (h x) -> p h x", x=128)[:, :, 0:64]
        src = psA[:, 0:CPC].rearrange("p (h x) -> p h x", x=64)
        nc.vector.tensor_copy(dst, src)
        yield

    def oproj_gen(self, qb, ssub, eb):
        nc = self.nc
        ssl = slice(ssub * 128, (ssub + 1) * 128)
        esl = slice(eb * 512, (eb + 1) * 512)
        if "cheapoproj" in ABLATE:
            oe = self.oe_pool.tile([128, 512], F32, tag="oe",
                                   name=f"oe{qb}_{ssub}_{eb}")
            nc.vector.memset(oe[:], 0.01)
            r0 = qb * 512 + ssub * 128
            nc.sync.dma_start(self.out_d[r0:r0 + 128, esl], oe[:])
            yield
            return
        psA = self.cd_pool.tile([128, 512], F32, tag="cd",
                                name=f"oA{qb}_{ssub}_{eb}")
        for p in range(2):
            st, sp = (p == 0), (p == 1)
            nc.tensor.matmul(psA[:], self.ot_tiles[(qb, p)][:, ssl],
                             self.ow_sb[p][:, esl], start=st, stop=sp)
            if p == 0:
                yield
        oe = self.oe_pool.tile([128, 512], F32, tag="oe", name=f"oe{qb}_{ssub}_{eb}")
        nc.vector.tensor_copy(oe[:], psA[:])
        r0 = qb * 512 + ssub * 128
        nc.sync.dma_start(self.out_d[r0:r0 + 128, esl], oe[:])
        yield

    def s_exp(self, qb, pair, c):
        nc = self.nc
        S = self.s_pool.tile([128, 1024], F32, tag="s", name=f"S{qb}_{pair}_{c}")
        csl = slice(c * 128, (c + 1) * 128)
        qsl = slice(qb * 512, (qb + 1) * 512)
        if S_MODE == "ktz":
            # lone K=128 matmuls against the zero-padded per-head kT tiles
            nc.tensor.matmul(S[:, 0:512], self.ktz[pair][0][:, csl],
                             self.qt[pair][:, qsl])
            nc.tensor.matmul(S[:, 512:1024], self.ktz[pair][1][:, csl],
                             self.qt[pair][:, qsl])
        else:
            nc.tensor.matmul(S[:, 0:512], self.kt[pair][0:64, csl],
                             self.qt[pair][0:64, qsl])
            nc.tensor.matmul(S[:, 512:1024], self.kt[pair][64:128, csl],
                             self.qt[pair][64:128, qsl])
        P = self.p_pool.tile([128, 1024], BF16, tag="p", name=f"P{qb}_{pair}_{c}")
        if "cheapexp" in ABLATE:
            nc.vector.tensor_copy(P[:], S[:])
        elif "memsetexp" in ABLATE:
            nc.vector.memset(P[:], 0.001)
        else:
            nc.scalar.activation(P[:], S[:], EXP, scale=0.125)
        return P

    def compute(self):
        nc = self.nc
        v_sb, ones = self.v_sb, self.ones
        cd_pool = self.cd_pool
        self.ot_tiles = {}

        feeder = Feeder()
        # ---- PE warmup: ~5us of dense matmuls flips the HAM clock gate to
        # K=8/8 (2.4 GHz); runs under the input-DMA wait so it's ~free ----
        if WARMUP:
            wups = cd_pool.tile([128, 512], F32, tag="cd", name="wups")
            for i in range(14):
                # col-tiled pair writing disjoint partition halves
                nc.tensor.matmul(wups[0:64, :], self.wu[:, 0:64], self.wu[:],
                                 start=True, stop=True, skip_group_check=True)
                nc.tensor.matmul(wups[64:128, :], self.wu[:, 64:128],
                                 self.wu[:], start=True, stop=True,
                                 skip_group_check=True)

        # ---- prefix: minimum projections to start attention ----
        feeder.run_all(self.kt_gen(0, 0))
        feeder.run_all(self.kt_gen(1, 0))
        for s in range(4):
            feeder.run_all(self.v_gen(s))
        feeder.run_all(self.qt_gen(0, 0))
        feeder.run_all(self.qt_gen(1, 0))

        # pair-0 chunk loop needs kT(p0, nb) before chunk 4*nb, v(s) before
        # chunk s (ordering validated against the pop(4) drain rate)
        feeder.add(self.kt_gen(0, 1))
        for s in range(4, 7):
            feeder.add(self.v_gen(s))
        feeder.add(self.kt_gen(0, 2))
        for s in range(7, 10):
            feeder.add(self.v_gen(s))
        feeder.add(self.kt_gen(0, 3))
        for s in range(10, 16):
            feeder.add(self.v_gen(s))
        feeder.add(self.kt_gen(1, 1))
        feeder.add(self.kt_gen(1, 2))
        feeder.add(self.kt_gen(1, 3))
        if "serialproj" in ABLATE:
            for qb in range(1, NQB):
                feeder.add(self.qt_gen(0, qb))
                feeder.add(self.qt_gen(1, qb))
            feeder.drain()

        if "noattn" in ABLATE:
            feeder.drain()
            for qb in range(NQB):
                for ssub in range(4):
                    for eb in range(2):
                        oe = self.oe_pool.tile([128, 512], F32, tag="oe",
                                               name=f"noe{qb}_{ssub}_{eb}")
                        nc.vector.memset(oe[:], 0.01)
                        r0 = qb * 512 + ssub * 128
                        nc.sync.dma_start(
                            self.out_d[r0:r0 + 128, eb * 512:(eb + 1) * 512],
                            oe[:])
            return

        # ---- attention main loop ----
        for qb in range(NQB):
            if qb + 1 < NQB and "serialproj" not in ABLATE:
                feeder.add(self.qt_gen(0, qb + 1))
                feeder.add(self.qt_gen(1, qb + 1))
            for pair in range(2):
                CA = self.c_pool.tile([128, 512], F32, tag="c",
                                      name=f"CA{qb}_{pair}")
                CB = self.d_pool.tile([128, 512], F32, tag="d",
                                      name=f"CB{qb}_{pair}")
                Ps = {0: self.s_exp(qb, pair, 0)}
                for c in range(NC_):
                    if c + 1 < NC_:
                        Ps[c + 1] = self.s_exp(qb, pair, c + 1)
                    P = Ps.pop(c)
                    st, sp = (c == 0), (c == NC_ - 1)
                    if CD_MODE == "fused":
                        # one M=128 matmul per head: [V_h | 1]^T P_h gives
                        # C_h in rows 0:64 and D_h (64x replicated) in rows
                        # 64:128 of one accumulator bank
                        blkA = (c * 4 + pair * 2) * 128
                        blkB = (c * 4 + pair * 2 + 1) * 128
                        nc.tensor.matmul(CA[:], v_sb[:, blkA:blkA + 128],
                                         P[:, 0:512], start=st, stop=sp)
                        nc.tensor.matmul(CB[:], v_sb[:, blkB:blkB + 128],
                                         P[:, 512:1024], start=st, stop=sp)
                    else:
                        off = c * CPC + pair * 128
                        nc.tensor.matmul(CA[0:64, :], v_sb[:, off:off + 64],
                                         P[:, 0:512], start=st, stop=sp,
                                         skip_group_check=True)
                        nc.tensor.matmul(CA[64:128, :],
                                         v_sb[:, off + 64:off + 128],
                                         P[:, 512:1024], start=st, stop=sp,
                                         skip_group_check=True)
                        if "nosum" not in ABLATE:
                            nc.tensor.matmul(CB[0:64, :], ones[:],
                                             P[:, 0:512], start=st, stop=sp,
                                             skip_group_check=True)
                            nc.tensor.matmul(CB[64:128, :], ones[:],
                                             P[:, 512:1024], start=st,
                                             stop=sp, skip_group_check=True)
                    # qb0/pair0 must drain the kT/v backlog fast enough to
                    # stay ahead of the chunk loop's own consumption; after
                    # that, spread the remaining filler work evenly so the PE
                    # stays busy (and the HAM clock-gate warm) through all of
                    # the attention phase
                    feeder.pop(4 if (qb == 0 and pair == 0) else 1)
                OT = self.ot_pool.tile([128, 512], BF16, tag="ot",
                                       name=f"OT{qb}_{pair}")
                if CD_MODE == "fused":
                    R = self.r_pool.tile([128, 512], F32, tag="r",
                                         name=f"R{qb}_{pair}")
                    # partition-shifted reads (validated on HW): align each
                    # head's replicated denominator with its C rows
                    nc.vector.reciprocal(R[0:64, :], CA[64:128, :])
                    nc.vector.reciprocal(R[64:128, :], CB[64:128, :])
                    nc.vector.tensor_mul(OT[0:64, :], R[0:64, :],
                                         CA[0:64, :])
                    nc.vector.tensor_mul(OT[64:128, :], R[64:128, :],
                                         CB[0:64, :])
                elif "nosum" in ABLATE:
                    nc.vector.tensor_copy(OT[:], CA[:])
                else:
                    R = self.r_pool.tile([128, 512], F32, tag="r",
                                         name=f"R{qb}_{pair}")
                    nc.vector.reciprocal(R[:], CB[:])
                    nc.vector.tensor_mul(OT[:], R[:], CA[:])
                self.ot_tiles[(qb, pair)] = OT
            for ssub in range(4):
                for eb in range(2):
                    feeder.add(self.oproj_gen(qb, ssub, eb))
            if "serialproj" in ABLATE:
                feeder.drain()
        feeder.drain()


_NC_CACHE = None


def _get_program():
    global _NC_CACHE
    if _NC_CACHE is None:
        _NC_CACHE = build_program()
    return _NC_CACHE


def shard_inputs(X, y, qW, qB, kW, kB, vW, vB, oW, oB):
    """Build the 8 per-core input maps (numpy, bf16 where appropriate)."""
    import ml_dtypes
    bf = ml_dtypes.bfloat16
    in_maps = []
    for core in range(N_CORES):
        b, g = divmod(core, 4)
        sl = slice(g * CPC, (g + 1) * CPC)
        in_maps.append({
            "xt": np.ascontiguousarray(np.asarray(X[b]).T).astype(bf),
            "yt": np.ascontiguousarray(np.asarray(y[b]).T).astype(bf),
            "qw": np.ascontiguousarray(np.asarray(qW)[:, sl]).astype(bf),
            "kw": np.ascontiguousarray(np.asarray(kW)[:, sl]).astype(bf),
            "vw": np.ascontiguousarray(np.asarray(vW)[:, sl]).astype(bf),
            "ow": np.ascontiguousarray(np.asarray(oW)[sl, :]).astype(bf),
            "qbias": np.asarray(qB)[sl].astype(np.float32),
            "kbias": np.asarray(kB)[sl].astype(np.float32),
        })
    return in_maps


def combine_outputs(partials, vB, oW, oB):
    """partials: list of 8 [SQ, D] fp32 arrays. Returns [B, SQ, D] fp32."""
    corr = (np.asarray(vB, np.float32) @ np.asarray(oW, np.float32)
            + np.asarray(oB, np.float32))
    out = np.empty((2, SQ, D), np.float32)
    for b in range(2):
        acc = partials[4 * b].astype(np.float32).copy()
        for g in range(1, 4):
            acc += partials[4 * b + g]
        out[b] = acc + corr
    return out


def kernel(X, y, qW, qB, kW, kB, vW, vB, oW, oB):
    from concourse.bass_utils import run_bass_kernel_spmd

    nc = _get_program()
    in_maps = shard_inputs(X, y, qW, qB, kW, kB, vW, vB, oW, oB)
    res = run_bass_kernel_spmd(nc, in_maps, list(range(N_CORES)))
    partials = [np.asarray(res.results[c]["out"], np.float32)
                for c in range(N_CORES)]
    return combine_outputs(partials, vB, oW, oB)

